# revision 53
# baseline (speedup 1.0000x reference)
"""DownscaleLabel Trainium2 kernel (v4: cast-DMA + 4x encodes + ramped PE).

Input:  label [8, 1024, 1024] int32, values in [-1, 6] (-1 = ignore).
Output: [8, 1, 64, 64] int32. Per 16x16 block: the dominant real class c
        (0..6) if its pixel count >= 192 (= 0.75 * 256), else -1.

Since 192 > 128, at most one class can reach the threshold, so
    out = -1 + sum_c (c+1) * [count_c >= 192]
needs no argmax or tie-breaking.

Per-core plan (one 1024x1024 image per NeuronCore, batch-sharded):

 - Input DMA: gpsimd SWDGE casts int32 -> int16 in flight, as 5 full-
   height column chunks (1024 descriptors each, so the fixed ~1us
   per-dma descriptor-gen cost stays off the critical path and the
   16 DMA engines run read-bound at full rate).
 - Encodes run entirely in DVE 4x perf mode (all operands int16):
     el16 = -640*x + 18176   (bf16 pattern 2^(15-5x): classes 2,1,0 at
            bits 5,10,15 after row-pool, class 3 at bit 0, ignore at 20)
     eh16 = 33152 - el16     (bf16 pattern 2^(5x-10): classes 3..6 at
            bits 5,10,15,20; x<=2 junk fractions)
 - PE p-state warmup: a few dummy bf16 matmuls on the consts bridge the
   DMA fill phase so the PE clock ramps (~2x) before real work arrives.
 - PE bf16 matmuls against block-diagonal ones row-pool 16 rows into
   PSUM fp32; the lo plane writes partitions 0:64 (array col-group 0x3)
   and the hi plane 64:128 (0xc), so the two planes co-schedule on the
   array.  Three column ranges [0,384), [384,768), [768,1024) keep the
   instruction count down while still draining early.
 - vi (ACT Relu) converts psum -> int32; fkw (DVE) extracts field pairs
   ((5,15) and (10,20), 10 bits apart); reds (DVE) col-pools 16 into
   pair-major rw [128, 2*64] (counts <= 256).
 - Threshold chain in three waves (one per column range): +320 per
   10-bit field makes bit 9/19 the count>=192 flag; shift/mask yields
   flag words; combine to one-hot V; multiply by per-partition magic M
   and shift/mask extracts the class weight -> acc bf16.
 - PE fold matmul sums partitions p and p+64, ACT subtracts 1, the
   int32 [64, 64] result DMAs out per wave on the SP ring.
"""

import sys

import numpy as np

_BASS_REPO = "/opt/trn_rl_repo"

H = W = 1024
SC = 16
TH = TW = 64
P = 128
NT = 8  # row-tiles of 128 rows
N_CORES = 8

# the last columns load raw int32 on the SP HWDGE ring (its descriptors
# exist ~2us before the gpsimd SWDGE ones, filling the early pipeline);
# the rest loads via gpsimd SWDGE with an int32->int16 cast
RAW_B = (896, 128)    # SP ring
CAST_CHUNKS = [(0, 256), (256, 256), (512, 256), (768, 128)]
# PE / downstream column ranges (c0, cw), in processing order
PE_RANGES = [(896, 128), (0, 256), (256, 256), (512, 256), (768, 128)]
# psum col offset per range (bank-aligned, both planes on partitions)
PRB = [0, 512, 1024, 1536, 2048]
PSCR = 2560           # scratch bank: PE p-state warmups + fold outputs
N_WARM = 8

PAIRMASK = 31 | (31 << 10)   # 0x7C1F
FLAG_C = 320 * 1025          # +320 per 10-bit field: bit9/bit19 = count>=192
M_MASK = 0x401               # flag word: bits 0,10 after >>9


def _ensure_path():
    if _BASS_REPO not in sys.path:
        sys.path.insert(0, _BASS_REPO)


def make_consts():
    """Host-side constant tensors fed as kernel inputs."""
    import ml_dtypes

    # Eight [128, 64] block-diagonal row-pooling patterns (pattern t places
    # tile t's 8 block-rows at out partitions 8t + k//16).  Columns 512:576
    # hold the fold pattern (k, k % 64) summing partitions p and p+64.
    poolw = np.zeros((P, 576), dtype=np.float32)
    k = np.arange(P)
    for t in range(NT):
        poolw[k, 64 * t + 8 * t + k // 16] = 1.0
    poolw[k, 512 + (k % 64)] = 1.0
    poolw = poolw.astype(ml_dtypes.bfloat16)

    # Magic multipliers: V has one-hot flag bits {0,3,10,13} for fields
    # 1..4; (V*M >> 13) mod 8 = class weight for the set field.
    # lo plane fields (1,2,3,4) -> weights (3,2,1,0); hi -> (4,5,6,7).
    mv = np.zeros((P, 1), dtype=np.float32)
    mv[:64, 0] = (3 << 13) | (2 << 10) | (1 << 3) | 0
    mv[64:, 0] = (4 << 13) | (5 << 10) | (6 << 3) | 7
    return poolw, mv


def emit_downscale(ctx, tc, out_ap, label_ap, poolw_ap, mv_ap):
    """Emit the per-core kernel body into TileContext tc."""
    _ensure_path()
    from concourse import mybir
    from concourse.alu_op_type import AluOpType as aop

    nc = tc.nc
    dt = mybir.dt

    cpool = ctx.enter_context(tc.tile_pool(name="consts", bufs=1))
    xpool = ctx.enter_context(tc.tile_pool(name="x", bufs=1))
    epool = ctx.enter_context(tc.tile_pool(name="e", bufs=1))
    ppool = ctx.enter_context(tc.tile_pool(name="psum", bufs=1, space="PSUM"))
    spool = ctx.enter_context(tc.tile_pool(name="small", bufs=1))

    # ---- consts (ACT HWDGE ring: lands early for the PE warmups) ----
    pw = cpool.tile([P, 576], dt.bfloat16)
    nc.scalar.dma_start(pw[:], poolw_ap)
    mv = cpool.tile([P, 1], dt.float32)
    nc.scalar.dma_start(mv[:], mv_ap)

    # ---- input DMA ----
    x32b = xpool.tile([P, NT * RAW_B[1]], dt.int32, tag="x32b")
    x16 = xpool.tile([P, NT * W], dt.int16, tag="x16")
    x_r = x16[:, :].rearrange("p (t c) -> p t c", t=NT)
    for (c0, cw) in CAST_CHUNKS:
        nc.gpsimd.dma_start(
            x_r[:, :, c0 : c0 + cw],
            label_ap[:, c0 : c0 + cw].rearrange("(t p) c -> p t c", p=P),
        )
    nc.sync.dma_start(
        x32b[:, :].rearrange("p (t c) -> p t c", t=NT),
        label_ap[:, RAW_B[0] : RAW_B[0] + RAW_B[1]].rearrange(
            "(t p) c -> p t c", p=P
        ),
    )

    # ---- tiles ----
    el16 = epool.tile([P, NT * W], dt.int16, tag="el16")
    eh16 = epool.tile([P, NT * W], dt.int16, tag="eh16")
    el_r = el16[:, :].rearrange("p (t c) -> p t c", t=NT)
    eh_r = eh16[:, :].rearrange("p (t c) -> p t c", t=NT)
    psum = ppool.tile([P, PSCR + 512], dt.float32)
    vi_t = spool.tile([P, W], dt.int32, tag="vi")
    fkw_t = spool.tile([P, 2 * W], dt.int32, tag="fkw")
    rw = spool.tile([P, 2 * TW], dt.int32, tag="rw")
    acc = spool.tile([P, TW], dt.bfloat16, tag="acc")
    resi = spool.tile([TH, TW], dt.int32, tag="resi")
    st = spool.tile([P, 2 * TW], dt.int32, tag="st")
    sm = spool.tile([P, 2 * TW], dt.int32, tag="sm")
    sv = spool.tile([P, TW], dt.int32, tag="sv")
    svm = spool.tile([P, TW], dt.int32, tag="svm")
    ssh = spool.tile([P, TW], dt.int32, tag="ssh")

    def warmups():
        # PE p-state warmup on the consts: keeps the array busy through the
        # DMA fill phase so the clock ramps before real matmuls arrive.
        for _ in range(N_WARM):
            nc.tensor.matmul(
                psum[0:TH, PSCR : PSCR + 512],
                pw[:, 0:TW],
                pw[:, 0:512],
                start=True,
                stop=True,
                skip_group_check=True,
            )

    def encode_raw(x32, c0, cw):
        # el from int32 (DVE 1x), eh from el (4x)
        v = nc.vector
        v.tensor_scalar(
            el_r[:, :, c0 : c0 + cw],
            x32[:, :].rearrange("p (t c) -> p t c", t=NT),
            -640, 18176, aop.mult, aop.add,
        )
        v.tensor_scalar(
            eh_r[:, :, c0 : c0 + cw], el_r[:, :, c0 : c0 + cw],
            -1, 33152, aop.mult, aop.add,
        )

    def encode_cast(ci):
        # both encodes all-int16: DVE 4x perf mode
        c0, cw = CAST_CHUNKS[ci]
        v = nc.vector
        v.tensor_scalar(
            el_r[:, :, c0 : c0 + cw], x_r[:, :, c0 : c0 + cw],
            -640, 18176, aop.mult, aop.add,
        )
        v.tensor_scalar(
            eh_r[:, :, c0 : c0 + cw], el_r[:, :, c0 : c0 + cw],
            -1, 33152, aop.mult, aop.add,
        )

    def mms(ri):
        c0, cw = PE_RANGES[ri]
        for t in range(NT):
            for plane, e_r in ((0, el_r), (1, eh_r)):
                nc.tensor.matmul(
                    psum[64 * plane : 64 * plane + 64, PRB[ri] : PRB[ri] + cw],
                    pw[:, 64 * t : 64 * (t + 1)],
                    e_r[:, t, c0 : c0 + cw].bitcast(dt.bfloat16),
                    start=(t == 0),
                    stop=(t == NT - 1),
                    skip_group_check=True,
                )

    def vi_op(ri):
        c0, cw = PE_RANGES[ri]
        nc.scalar.activation(
            vi_t[:, c0 : c0 + cw],
            psum[:, PRB[ri] : PRB[ri] + cw],
            mybir.ActivationFunctionType.Relu,
            bias=0.0,
            scale=1.0,
        )

    def fkws(ri):
        c0, cw = PE_RANGES[ri]
        for j, shift in enumerate((5, 10)):
            nc.vector.tensor_scalar(
                fkw_t[:, W * j + c0 : W * j + c0 + cw],
                vi_t[:, c0 : c0 + cw],
                shift,
                PAIRMASK,
                aop.logical_shift_right,
                aop.bitwise_and,
            )

    def reds(ri):
        c0, cw = PE_RANGES[ri]
        for j in range(2):
            with nc.allow_low_precision(reason="small int counts, exact"):
                nc.vector.tensor_reduce(
                    rw[:, TW * j + c0 // SC : TW * j + (c0 + cw) // SC],
                    fkw_t[:, W * j + c0 : W * j + c0 + cw].rearrange(
                        "p (b s) -> p b s", s=SC
                    ),
                    mybir.AxisListType.X,
                    aop.add,
                )

    def smalls(b0, b1):
        # threshold pass over block-columns [b0, b1)
        v = nc.vector

        def w2(t):
            return t[:, :].rearrange("p (j b) -> p j b", j=2)[:, :, b0:b1]

        v.tensor_scalar(w2(st), w2(rw), FLAG_C, None, aop.add)
        v.tensor_scalar(
            w2(sm), w2(st), 9, M_MASK, aop.logical_shift_right, aop.bitwise_and
        )
        # V = m_pair0 + 8 * m_pair1 -> one-hot bits {0,3,10,13}
        v.scalar_tensor_tensor(
            sv[:, b0:b1], sm[:, TW + b0 : TW + b1], 8, sm[:, b0:b1],
            aop.mult, aop.add,
        )
        v.tensor_scalar(svm[:, b0:b1], sv[:, b0:b1], mv[:, 0:1], None, aop.mult)
        v.tensor_scalar(
            ssh[:, b0:b1], svm[:, b0:b1], 13, 7,
            aop.logical_shift_right, aop.bitwise_and,
        )
        v.tensor_copy(acc[:, b0:b1], ssh[:, b0:b1])
        # fold: out[p, b] = acc[p, b] + acc[p+64, b] - 1  (values <= 7, exact)
        nc.tensor.matmul(
            psum[0:TH, PSCR + b0 : PSCR + b1], pw[:, 512:576], acc[:, b0:b1],
            start=True, stop=True, skip_group_check=True,
        )
        nc.scalar.activation(
            resi[:, b0:b1],
            psum[0:TH, PSCR + b0 : PSCR + b1],
            mybir.ActivationFunctionType.Copy,
            bias=-1.0,
            scale=1.0,
        )
        nc.sync.dma_start(out_ap[:, b0:b1], resi[:, b0:b1])

    def down(ri):
        fkws(ri)
        reds(ri)
        c0, cw = PE_RANGES[ri]
        smalls(c0 // SC, (c0 + cw) // SC)

    # ---- pipeline emission ----
    # (interleaved so each engine's in-order program never stalls long:
    #  DVE fills its data-wait gaps with the previous range's downstream)
    warmups()
    encode_raw(x32b, *RAW_B)   # SP raw chunk lands first
    mms(0)
    vi_op(0)
    encode_cast(0)
    down(0)                    # rB downstream
    mms(1)
    vi_op(1)
    encode_cast(1)
    down(1)                    # r0 downstream
    mms(2)
    vi_op(2)
    encode_cast(2)
    down(2)
    mms(3)
    vi_op(3)
    encode_cast(3)             # last (small) cast chunk
    down(3)
    mms(4)
    vi_op(4)
    down(4)


def _split_multi_waits(nc):
    """This toolchain's walrus codegen accepts at most ONE semaphore wait per
    engine instruction (two on EventSemaphore).  The Tile scheduler sometimes
    emits more; spill the extras onto same-engine NoOp carriers inserted just
    before the instruction (engines dispatch in order, so the carrier's wait
    is satisfied before the instruction issues -- semantics preserved)."""
    _ensure_path()
    from concourse import mybir

    for func in nc.m.functions:
        for blk in func.blocks:
            insts = blk.instructions
            out = []
            changed = False
            for ins in insts:
                si = ins.sync_info
                cap = 2 if isinstance(ins, mybir.InstEventSemaphore) else 1
                if si and si.on_wait and len(si.on_wait) > cap:
                    waits = list(si.on_wait)
                    for w in waits[:-cap]:
                        out.append(
                            mybir.InstNoOp(
                                name=nc.get_next_instruction_name(),
                                engine=ins.engine,
                                sync_info=mybir.SyncInfo(on_wait=[w], on_update=[]),
                                bass_nofuse=True,
                            )
                        )
                    si.on_wait = waits[-cap:]
                    changed = True
                out.append(ins)
            if changed:
                blk.instructions = out


def _install_ntff_hook():
    """Provide antenv.axon_hooks + the ctypes NTFF profile hook when the
    agent image lacks them (mirrors trn_agent_boot.trn_boot section 6)."""
    import contextlib
    import ctypes
    import types

    try:
        from antenv.axon_hooks import get_axon_ntff_profile_hook  # noqa: F401

        return
    except ImportError:
        pass
    _ensure_path()
    import antenv

    so_path = "/opt/axon/libaxon_pjrt.so"
    try:
        lib = ctypes.CDLL(so_path)
    except OSError:
        return
    if not hasattr(lib, "axon_start_nrt_profile"):
        return
    lib.axon_start_nrt_profile.argtypes = [
        ctypes.POINTER(ctypes.c_int64),
        ctypes.c_size_t,
    ]
    lib.axon_start_nrt_profile.restype = ctypes.c_int64
    lib.axon_stop_nrt_profile.argtypes = [ctypes.c_char_p]
    lib.axon_stop_nrt_profile.restype = ctypes.c_int64

    @contextlib.contextmanager
    def _hook(output_dir, device_ids):
        import jax

        jax.devices()
        if device_ids:
            ids = (ctypes.c_int64 * len(device_ids))(*device_ids)
            rc = lib.axon_start_nrt_profile(ids, len(device_ids))
        else:
            rc = lib.axon_start_nrt_profile(None, 0)
        if rc != 0:
            raise RuntimeError(f"axon_start_nrt_profile rc={rc}")
        try:
            yield
        finally:
            n = lib.axon_stop_nrt_profile(str(output_dir).encode())
            print(f"ntff profile: {n} file(s) written to {output_dir}", file=sys.stderr)

    mod = types.ModuleType("antenv.axon_hooks")
    _h = [_hook]
    mod.set_axon_ntff_profile_hook = lambda h: _h.__setitem__(0, h)
    mod.get_axon_ntff_profile_hook = lambda: _h[0]
    sys.modules["antenv.axon_hooks"] = mod
    antenv.axon_hooks = mod

    # upload_artifacts pushes the NEFF dir to a cloud bucket; keep local.
    from concourse import bass_utils as _bu

    _bu.upload_artifacts = lambda tmpdir: tmpdir


_NC_CACHE = None


def _build_nc(split_waits=True):
    global _NC_CACHE
    if _NC_CACHE is not None:
        return _NC_CACHE
    _ensure_path()
    from contextlib import ExitStack

    import concourse.bass as bass
    import concourse.tile as tile
    from concourse import mybir

    dt = mybir.dt
    nc = bass.Bass("TRN2", target_bir_lowering=False, debug=False)
    label = nc.dram_tensor("label", [H, W], dt.int32, kind="ExternalInput").ap()
    poolw = nc.dram_tensor("poolw", [P, 576], dt.bfloat16, kind="ExternalInput").ap()
    mvt = nc.dram_tensor("mv", [P, 1], dt.float32, kind="ExternalInput").ap()
    out = nc.dram_tensor("out", [TH, TW], dt.int32, kind="ExternalOutput").ap()
    with tile.TileContext(nc) as tc:
        with ExitStack() as ctx:
            emit_downscale(ctx, tc, out, label, poolw, mvt)
    if split_waits:
        _split_multi_waits(nc)
        _NC_CACHE = nc
    return nc


def run_on_hw(label, trace=False):
    """Run on the 8 NeuronCores; returns (out [8,1,64,64] int32, exec_time_ns)."""
    _ensure_path()
    from concourse.bass_utils import run_bass_kernel_spmd

    if trace:
        _install_ntff_hook()
    nc = _build_nc()
    poolw, mv = make_consts()
    label = np.ascontiguousarray(label, dtype=np.int32)
    in_maps = [
        {"label": label[i], "poolw": poolw, "mv": mv} for i in range(N_CORES)
    ]
    r = run_bass_kernel_spmd(nc, in_maps, core_ids=list(range(N_CORES)), trace=trace)
    outs = np.stack([r.results[i]["out"] for i in range(N_CORES)])
    return outs.reshape(8, 1, TH, TW).astype(np.int32), r.exec_time_ns


def kernel(label):
    out, _ = run_on_hw(label, trace=False)
    return out


# revision 55
# speedup vs baseline: 1.4510x; 1.4510x over previous
"""DownscaleLabel Trainium2 kernel (v2: column-block streaming, cast-DMA).

Input:  label [8, 1024, 1024] int32, values in [-1, 6] (-1 = ignore).
Output: [8, 1, 64, 64] int32. Per 16x16 block: the dominant real class c
        (0..6) if its pixel count >= 192 (= 0.75 * 256), else -1.

Since 192 > 128, at most one class can reach the threshold, so
    out = -1 + sum_c (c+1) * [count_c >= 192]
needs no argmax or tie-breaking.

Per-core plan (one 1024x1024 image per NeuronCore, batch-sharded):

The image is processed as 3 COLUMN blocks of [512, 384, 128] columns; each
block's histograms finalize when its matmuls complete, so early blocks'
downstream overlaps the DMA stream of later blocks.

 - Input DMA: SWDGE (gpsimd) transfers CAST int32 -> int16 in flight, so
   SBUF holds int16 labels; all 9 chunk transfers queue on the single
   SWDGE ring early and drain in order at full rate.
 - Encodings read int16 and write int16, making DVE tensor_scalar eligible
   for its 4x mode (~0.30 ns/col measured vs 0.9 on ACT):
     el: bf16 bit pattern of 2^(15-5x): classes 2,1,0 in 5-bit fields
         1..3, ignore count in field 4; x=3 -> field 0; x>=4 fractions.
     eh: 2^(5x-10): classes 3..6 in fields 1..4; x<=2 junk/fractions.
 - PE matmuls against block-diagonal ones row-pool 16 rows into PSUM fp32
   (fields <= 16).  BOTH planes share one PSUM bank per block at the same
   columns (lo plane partitions 0:64, hi 64:128; per-partition pending-
   zero makes the interleaved accumulation groups safe), so the psum->int
   cast (vi) is a single [128, w] op per block.
 - fkw extracts field pairs ((1,3), (2,4), 10 bits apart); tensor_reduce
   col-pools 16 into pair-major rw [128, 2*64] (counts <= 256, bits 9/19
   free).
 - Threshold chain (6 DVE ops, single pass): +320 per 10-bit field makes
   bit 9/19 the count>=192 flag; one shift/mask yields flag words for both
   pairs at bits {0,10}; combine to one-hot V (bits {0,3,10,13}); multiply
   by per-partition magic M and a shift/mask extracts the class weight
   (class id + 1, or 0) -> acc bf16.
 - PE fold matmul sums partitions p and p+64 (<= one weight nonzero), ACT
   subtracts 1, result [64, 64] int32 DMAs out.
"""

import sys

import numpy as np

_BASS_REPO = "/opt/trn_rl_repo"

H = W = 1024
SC = 16
TH = TW = 64
P = 128
NT = H // P  # 8 row-tiles
N_CORES = 8

# Column blocks and their DMA chunks (row-tile ranges per transfer).
W_B = [512, 384, 128]
OFF_B = [0, 512, 896]
CHUNKS = [
    [(0, 2), (2, 4), (4, 6), (6, 8)],
    [(0, 4), (4, 8)],
    [(0, 8)],
]
# flat stream order of (block, t0, t1)
STREAM = [(b, t0, t1) for b in range(3) for (t0, t1) in CHUNKS[b]]
NB_B = [w // SC for w in W_B]  # [32, 24, 8]
BOFF = [0, 32, 56]
PBANK = [0, 512, 1024]         # psum col offset per block (bank-aligned)
PFOLD = 1536
PSCR = 2048                    # scratch bank for PE p-state warmups
N_WARM = 10

PAIRMASK = 31 | (31 << 10)   # 0x7C1F
FLAG_C = 320 * 1025          # +320 per 10-bit field: bit9/bit19 = count>=192
M_MASK = 0x401               # flag word: bits 0,10 after >>9


def _ensure_path():
    if _BASS_REPO not in sys.path:
        sys.path.insert(0, _BASS_REPO)


def make_consts():
    """Host-side constant tensors fed as kernel inputs."""
    import ml_dtypes

    # Eight [128, 64] block-diagonal row-pooling patterns (pattern t places
    # tile t's 8 block-rows at out partitions 8t + k//16).  Columns 512:576
    # hold the fold pattern (k, k % 64) summing partitions p and p+64.
    poolw = np.zeros((P, 576), dtype=np.float32)
    k = np.arange(P)
    for t in range(NT):
        poolw[k, 64 * t + 8 * t + k // 16] = 1.0
    poolw[k, 512 + (k % 64)] = 1.0
    poolw = poolw.astype(ml_dtypes.bfloat16)

    # Magic multipliers: V has one-hot flag bits {0,3,10,13} for fields
    # 1..4; (V*M >> 13) mod 8 = class weight for the set field.
    # lo plane fields (1,2,3,4) -> weights (3,2,1,0); hi -> (4,5,6,7).
    mv = np.zeros((P, 1), dtype=np.float32)
    mv[:64, 0] = (3 << 13) | (2 << 10) | (1 << 3) | 0
    mv[64:, 0] = (4 << 13) | (5 << 10) | (6 << 3) | 7
    return poolw, mv


def emit_downscale(ctx, tc, out_ap, label_ap, poolw_ap, mv_ap):
    """Emit the per-core kernel body into TileContext tc."""
    _ensure_path()
    from concourse import mybir
    from concourse.alu_op_type import AluOpType as aop

    nc = tc.nc
    dt = mybir.dt

    cpool = ctx.enter_context(tc.tile_pool(name="consts", bufs=1))
    xpool = ctx.enter_context(tc.tile_pool(name="x", bufs=1))
    epool = ctx.enter_context(tc.tile_pool(name="e", bufs=1))
    ppool = ctx.enter_context(tc.tile_pool(name="psum", bufs=1, space="PSUM"))
    spool = ctx.enter_context(tc.tile_pool(name="small", bufs=1))

    # ---- consts (scalar HWDGE ring, issued at t=0) ----
    pw = cpool.tile([P, 576], dt.bfloat16)
    nc.scalar.dma_start(pw[:], poolw_ap)
    mv = cpool.tile([P, 1], dt.float32)
    nc.scalar.dma_start(mv[:], mv_ap)

    # PE p-state warmup: dummy bf16 matmuls on the consts keep the array
    # busy through the DMA fill phase so the clock ramps early.
    def warmups():
        for _ in range(N_WARM):
            nc.tensor.matmul(
                psum[0:64, PSCR : PSCR + 512],
                pw[:, 0:64],
                pw[:, 0:512],
                start=True,
                stop=True,
                skip_group_check=True,
            )

    # ---- input DMA: SWDGE cast int32 -> int16, stream order ----
    xs = [
        xpool.tile([P, NT * w], dt.int16, name=f"x{b}", tag=f"x{b}")
        for b, w in enumerate(W_B)
    ]
    for (b, t0, t1) in STREAM:
        w = W_B[b]
        off = OFF_B[b]
        nc.gpsimd.dma_start(
            xs[b][:, t0 * w : t1 * w].rearrange("p (t c) -> p t c", t=t1 - t0),
            label_ap[P * t0 : P * t1, off : off + w].rearrange(
                "(t p) c -> p t c", p=P
            ),
        )

    # ---- tiles ----
    els = [
        epool.tile([P, NT * w], dt.int16, name=f"el{b}", tag=f"el{b}")
        for b, w in enumerate(W_B)
    ]
    ehs = [
        epool.tile([P, NT * w], dt.int16, name=f"eh{b}", tag=f"eh{b}")
        for b, w in enumerate(W_B)
    ]
    psum = ppool.tile([P, PSCR + 512], dt.float32)
    vis = [
        spool.tile([P, w], dt.int32, name=f"vi{b}", tag=f"vi{b}")
        for b, w in enumerate(W_B)
    ]
    fkw = [
        spool.tile([P, 2 * w], dt.int32, name=f"fkw{b}", tag=f"fkw{b}")
        for b, w in enumerate(W_B)
    ]
    rw = spool.tile([P, 2 * TW], dt.int32, tag="rw")
    acc = spool.tile([P, TW], dt.bfloat16, tag="acc")
    resi = spool.tile([TH, TW], dt.int32, tag="resi")
    st = spool.tile([P, 2 * TW], dt.int32, tag="st")   # smalls scratch t
    sm = spool.tile([P, 2 * TW], dt.int32, tag="sm")   # smalls flag words
    sv = spool.tile([P, TW], dt.int32, tag="sv")       # one-hot V
    svm = spool.tile([P, TW], dt.int32, tag="svm")     # V * M
    ssh = spool.tile([P, TW], dt.int32, tag="ssh")     # VM >> 13

    def encode(plane, b, t0, t1, engine):
        w = W_B[b]
        e = (els if plane == 0 else ehs)[b]
        scale, bias = (-640, 18176) if plane == 0 else (640, 14976)
        xsrc = xs[b][:, t0 * w : t1 * w]
        nc.vector.tensor_scalar(
            e[:, t0 * w : t1 * w],
            xsrc,
            scale,
            bias,
            aop.mult,
            aop.add,
        )

    def mms(b, t0, t1):
        w = W_B[b]
        for t in range(t0, t1):
            for plane, e in ((0, els[b]), (1, ehs[b])):
                base = 64 * plane
                nc.tensor.matmul(
                    psum[base : base + 64, PBANK[b] : PBANK[b] + w],
                    pw[:, 64 * t : 64 * (t + 1)],
                    e[:, t * w : (t + 1) * w].bitcast(dt.bfloat16),
                    start=(t == 0),
                    stop=(t == NT - 1),
                    skip_group_check=True,
                )

    def vi_op(b, engine):
        w = W_B[b]
        nc.scalar.activation(
            vis[b][:],
            psum[:, PBANK[b] : PBANK[b] + w],
            mybir.ActivationFunctionType.Relu,
            bias=0.0,
            scale=1.0,
        )

    def fkws(b):
        w = W_B[b]
        for j, shift in enumerate((5, 10)):
            nc.vector.tensor_scalar(
                fkw[b][:, w * j : w * (j + 1)],
                vis[b][:],
                shift,
                PAIRMASK,
                aop.logical_shift_right,
                aop.bitwise_and,
            )

    def reds(b):
        w = W_B[b]
        nb = NB_B[b]
        for j in range(2):
            with nc.allow_low_precision(reason="small int counts, exact"):
                nc.vector.tensor_reduce(
                    rw[:, TW * j + BOFF[b] : TW * j + BOFF[b] + nb],
                    fkw[b][:, w * j : w * (j + 1)].rearrange(
                        "p (x s) -> p x s", s=SC
                    ),
                    mybir.AxisListType.X,
                    aop.add,
                )

    def smalls(b0, b1):
        # threshold pass over block-columns [b0, b1), fold + output dma
        v = nc.vector

        def w2(t):
            return t[:, :].rearrange("p (j b) -> p j b", j=2)[:, :, b0:b1]

        v.tensor_scalar(w2(st), w2(rw), FLAG_C, None, aop.add)
        v.tensor_scalar(
            w2(sm), w2(st), 9, M_MASK, aop.logical_shift_right, aop.bitwise_and
        )
        # V = m_pair0 + 8 * m_pair1 -> one-hot bits {0,3,10,13}
        v.scalar_tensor_tensor(
            sv[:, b0:b1], sm[:, TW + b0 : TW + b1], 8, sm[:, b0:b1],
            aop.mult, aop.add,
        )
        v.tensor_scalar(svm[:, b0:b1], sv[:, b0:b1], mv[:, 0:1], None, aop.mult)
        v.tensor_scalar(
            ssh[:, b0:b1], svm[:, b0:b1], 13, 7,
            aop.logical_shift_right, aop.bitwise_and,
        )
        v.tensor_copy(acc[:, b0:b1], ssh[:, b0:b1])
        # fold: out[p, b] = acc[p, b] + acc[p+64, b] - 1  (values <= 7, exact)
        nc.tensor.matmul(
            psum[0:64, PFOLD + b0 : PFOLD + b1], pw[:, 512:576], acc[:, b0:b1],
            start=True, stop=True, skip_group_check=True,
        )
        nc.scalar.activation(
            resi[:, b0:b1],
            psum[0:64, PFOLD + b0 : PFOLD + b1],
            mybir.ActivationFunctionType.Copy,
            bias=-1.0,
            scale=1.0,
        )
        nc.sync.dma_start(out_ap[:, b0:b1], resi[:, b0:b1])

    # ---- pipeline emission ----
    warmups()
    for (t0, t1) in CHUNKS[0]:
        encode(0, 0, t0, t1, "dve")
        encode(1, 0, t0, t1, "dve")
        mms(0, t0, t1)
    encode(0, 1, 0, 4, "dve")
    encode(1, 1, 0, 4, "dve")
    mms(1, 0, 4)
    vi_op(0, "act")
    fkws(0)
    reds(0)
    encode(1, 1, 4, 8, "dve")
    encode(0, 1, 4, 8, "dve")
    mms(1, 4, 8)
    encode(1, 2, 0, 8, "dve")
    encode(0, 2, 0, 8, "dve")
    mms(2, 0, 8)
    vi_op(1, "act")
    fkws(1)
    reds(1)
    smalls(0, 56)
    vi_op(2, "act")
    fkws(2)
    reds(2)
    smalls(56, TW)


def _split_multi_waits(nc):
    """This toolchain's walrus codegen accepts at most ONE semaphore wait per
    engine instruction (two on EventSemaphore).  The Tile scheduler sometimes
    emits more; spill the extras onto same-engine NoOp carriers inserted just
    before the instruction (engines dispatch in order, so the carrier's wait
    is satisfied before the instruction issues -- semantics preserved)."""
    _ensure_path()
    from concourse import mybir

    for func in nc.m.functions:
        for blk in func.blocks:
            insts = blk.instructions
            out = []
            changed = False
            for ins in insts:
                si = ins.sync_info
                cap = 2 if isinstance(ins, mybir.InstEventSemaphore) else 1
                if si and si.on_wait and len(si.on_wait) > cap:
                    waits = list(si.on_wait)
                    for w in waits[:-cap]:
                        out.append(
                            mybir.InstNoOp(
                                name=nc.get_next_instruction_name(),
                                engine=ins.engine,
                                sync_info=mybir.SyncInfo(on_wait=[w], on_update=[]),
                                bass_nofuse=True,
                            )
                        )
                    si.on_wait = waits[-cap:]
                    changed = True
                out.append(ins)
            if changed:
                blk.instructions = out


def _install_ntff_hook():
    """Provide antenv.axon_hooks + the ctypes NTFF profile hook when the
    agent image lacks them (mirrors trn_agent_boot.trn_boot section 6)."""
    import contextlib
    import ctypes
    import types

    try:
        from antenv.axon_hooks import get_axon_ntff_profile_hook  # noqa: F401

        return
    except ImportError:
        pass
    _ensure_path()
    import antenv

    so_path = "/opt/axon/libaxon_pjrt.so"
    try:
        lib = ctypes.CDLL(so_path)
    except OSError:
        return
    if not hasattr(lib, "axon_start_nrt_profile"):
        return
    lib.axon_start_nrt_profile.argtypes = [
        ctypes.POINTER(ctypes.c_int64),
        ctypes.c_size_t,
    ]
    lib.axon_start_nrt_profile.restype = ctypes.c_int64
    lib.axon_stop_nrt_profile.argtypes = [ctypes.c_char_p]
    lib.axon_stop_nrt_profile.restype = ctypes.c_int64

    @contextlib.contextmanager
    def _hook(output_dir, device_ids):
        import jax

        jax.devices()
        if device_ids:
            ids = (ctypes.c_int64 * len(device_ids))(*device_ids)
            rc = lib.axon_start_nrt_profile(ids, len(device_ids))
        else:
            rc = lib.axon_start_nrt_profile(None, 0)
        if rc != 0:
            raise RuntimeError(f"axon_start_nrt_profile rc={rc}")
        try:
            yield
        finally:
            n = lib.axon_stop_nrt_profile(str(output_dir).encode())
            print(f"ntff profile: {n} file(s) written to {output_dir}", file=sys.stderr)

    mod = types.ModuleType("antenv.axon_hooks")
    _h = [_hook]
    mod.set_axon_ntff_profile_hook = lambda h: _h.__setitem__(0, h)
    mod.get_axon_ntff_profile_hook = lambda: _h[0]
    sys.modules["antenv.axon_hooks"] = mod
    antenv.axon_hooks = mod

    # upload_artifacts pushes the NEFF dir to a cloud bucket; keep local.
    from concourse import bass_utils as _bu

    _bu.upload_artifacts = lambda tmpdir: tmpdir


_NC_CACHE = None


def _build_nc(split_waits=True):
    global _NC_CACHE
    if _NC_CACHE is not None:
        return _NC_CACHE
    _ensure_path()
    from contextlib import ExitStack

    import concourse.bass as bass
    import concourse.tile as tile
    from concourse import mybir

    dt = mybir.dt
    nc = bass.Bass("TRN2", target_bir_lowering=False, debug=False)
    label = nc.dram_tensor("label", [H, W], dt.int32, kind="ExternalInput").ap()
    poolw = nc.dram_tensor("poolw", [P, 576], dt.bfloat16, kind="ExternalInput").ap()
    mvt = nc.dram_tensor("mv", [P, 1], dt.float32, kind="ExternalInput").ap()
    out = nc.dram_tensor("out", [TH, TW], dt.int32, kind="ExternalOutput").ap()
    with tile.TileContext(nc) as tc:
        with ExitStack() as ctx:
            emit_downscale(ctx, tc, out, label, poolw, mvt)
    if split_waits:
        _split_multi_waits(nc)
        _NC_CACHE = nc
    return nc


def run_on_hw(label, trace=False):
    """Run on the 8 NeuronCores; returns (out [8,1,64,64] int32, exec_time_ns)."""
    _ensure_path()
    from concourse.bass_utils import run_bass_kernel_spmd

    if trace:
        _install_ntff_hook()
    nc = _build_nc()
    poolw, mv = make_consts()
    label = np.ascontiguousarray(label, dtype=np.int32)
    in_maps = [
        {"label": label[i], "poolw": poolw, "mv": mv} for i in range(N_CORES)
    ]
    r = run_bass_kernel_spmd(nc, in_maps, core_ids=list(range(N_CORES)), trace=trace)
    outs = np.stack([r.results[i]["out"] for i in range(N_CORES)])
    return outs.reshape(8, 1, TH, TW).astype(np.int32), r.exec_time_ns


def kernel(label):
    out, _ = run_on_hw(label, trace=False)
    return out



# revision 56
# speedup vs baseline: 1.5180x; 1.0462x over previous
"""DownscaleLabel Trainium2 kernel (v2: column-block streaming, cast-DMA).

Input:  label [8, 1024, 1024] int32, values in [-1, 6] (-1 = ignore).
Output: [8, 1, 64, 64] int32. Per 16x16 block: the dominant real class c
        (0..6) if its pixel count >= 192 (= 0.75 * 256), else -1.

Since 192 > 128, at most one class can reach the threshold, so
    out = -1 + sum_c (c+1) * [count_c >= 192]
needs no argmax or tie-breaking.

Per-core plan (one 1024x1024 image per NeuronCore, batch-sharded):

The image is processed as 3 COLUMN blocks of [512, 384, 128] columns; each
block's histograms finalize when its matmuls complete, so early blocks'
downstream overlaps the DMA stream of later blocks.

 - Input DMA: SWDGE (gpsimd) transfers CAST int32 -> int16 in flight, so
   SBUF holds int16 labels; all 9 chunk transfers queue on the single
   SWDGE ring early and drain in order at full rate.
 - Encodings read int16 and write int16, making DVE tensor_scalar eligible
   for its 4x mode (~0.30 ns/col measured vs 0.9 on ACT):
     el: bf16 bit pattern of 2^(15-5x): classes 2,1,0 in 5-bit fields
         1..3, ignore count in field 4; x=3 -> field 0; x>=4 fractions.
     eh: 2^(5x-10): classes 3..6 in fields 1..4; x<=2 junk/fractions.
 - PE matmuls against block-diagonal ones row-pool 16 rows into PSUM fp32
   (fields <= 16).  BOTH planes share one PSUM bank per block at the same
   columns (lo plane partitions 0:64, hi 64:128; per-partition pending-
   zero makes the interleaved accumulation groups safe), so the psum->int
   cast (vi) is a single [128, w] op per block.
 - fkw extracts field pairs ((1,3), (2,4), 10 bits apart); tensor_reduce
   col-pools 16 into pair-major rw [128, 2*64] (counts <= 256, bits 9/19
   free).
 - Threshold chain (6 DVE ops, single pass): +320 per 10-bit field makes
   bit 9/19 the count>=192 flag; one shift/mask yields flag words for both
   pairs at bits {0,10}; combine to one-hot V (bits {0,3,10,13}); multiply
   by per-partition magic M and a shift/mask extracts the class weight
   (class id + 1, or 0) -> acc bf16.
 - PE fold matmul sums partitions p and p+64 (<= one weight nonzero), ACT
   subtracts 1, result [64, 64] int32 DMAs out.
"""

import sys

import numpy as np

_BASS_REPO = "/opt/trn_rl_repo"

H = W = 1024
SC = 16
TH = TW = 64
P = 128
NT = H // P  # 8 row-tiles
N_CORES = 8

# Column blocks and their DMA chunks (row-tile ranges per transfer).
W_B = [512, 384, 128]
OFF_B = [0, 512, 896]
CHUNKS = [
    [(0, 2), (2, 4), (4, 6), (6, 8)],
    [(0, 4), (4, 8)],
    [(0, 8)],
]
# flat stream order of (block, t0, t1)
STREAM = [(b, t0, t1) for b in range(3) for (t0, t1) in CHUNKS[b]]
NB_B = [w // SC for w in W_B]  # [32, 24, 8]
BOFF = [0, 32, 56]
PBANK = [0, 512, 1024]         # psum col offset per block (bank-aligned)
PFOLD = 1536
PSCR = 2048                    # scratch bank for PE p-state warmups
N_WARM = 10

PAIRMASK = 31 | (31 << 10)   # 0x7C1F
FLAG_C = 320 * 1025          # +320 per 10-bit field: bit9/bit19 = count>=192
M_MASK = 0x401               # flag word: bits 0,10 after >>9


def _ensure_path():
    if _BASS_REPO not in sys.path:
        sys.path.insert(0, _BASS_REPO)


def make_consts():
    """Host-side constant tensors fed as kernel inputs."""
    import ml_dtypes

    # Eight [128, 64] block-diagonal row-pooling patterns (pattern t places
    # tile t's 8 block-rows at out partitions 8t + k//16).  Columns 512:576
    # hold the fold pattern (k, k % 64) summing partitions p and p+64.
    poolw = np.zeros((P, 576), dtype=np.float32)
    k = np.arange(P)
    for t in range(NT):
        poolw[k, 64 * t + 8 * t + k // 16] = 1.0
    poolw[k, 512 + (k % 64)] = 1.0
    poolw = poolw.astype(ml_dtypes.bfloat16)

    # Magic multipliers: V has one-hot flag bits {0,3,10,13} for fields
    # 1..4; (V*M >> 13) mod 8 = class weight for the set field.
    # lo plane fields (1,2,3,4) -> weights (3,2,1,0); hi -> (4,5,6,7).
    mv = np.zeros((P, 1), dtype=np.float32)
    mv[:64, 0] = (3 << 13) | (2 << 10) | (1 << 3) | 0
    mv[64:, 0] = (4 << 13) | (5 << 10) | (6 << 3) | 7
    return poolw, mv


def emit_downscale(ctx, tc, out_ap, label_ap, poolw_ap, mv_ap):
    """Emit the per-core kernel body into TileContext tc."""
    _ensure_path()
    from concourse import mybir
    from concourse.alu_op_type import AluOpType as aop

    nc = tc.nc
    dt = mybir.dt

    cpool = ctx.enter_context(tc.tile_pool(name="consts", bufs=1))
    xpool = ctx.enter_context(tc.tile_pool(name="x", bufs=1))
    epool = ctx.enter_context(tc.tile_pool(name="e", bufs=1))
    ppool = ctx.enter_context(tc.tile_pool(name="psum", bufs=1, space="PSUM"))
    spool = ctx.enter_context(tc.tile_pool(name="small", bufs=1))

    # ---- consts (scalar HWDGE ring, issued at t=0) ----
    pw = cpool.tile([P, 576], dt.bfloat16)
    nc.scalar.dma_start(pw[:], poolw_ap)
    mv = cpool.tile([P, 1], dt.float32)
    nc.scalar.dma_start(mv[:], mv_ap)

    # PE p-state warmup: dummy bf16 matmuls on the consts keep the array
    # busy through the DMA fill phase so the clock ramps early.
    def warmups():
        for _ in range(N_WARM):
            nc.tensor.matmul(
                psum[0:64, PSCR : PSCR + 512],
                pw[:, 0:64],
                pw[:, 0:512],
                start=True,
                stop=True,
                skip_group_check=True,
            )

    # ---- input DMA: SWDGE cast int32 -> int16, stream order ----
    xs = [
        xpool.tile([P, NT * w], dt.int16, name=f"x{b}", tag=f"x{b}")
        for b, w in enumerate(W_B)
    ]
    for (b, t0, t1) in STREAM:
        w = W_B[b]
        off = OFF_B[b]
        nc.gpsimd.dma_start(
            xs[b][:, t0 * w : t1 * w].rearrange("p (t c) -> p t c", t=t1 - t0),
            label_ap[P * t0 : P * t1, off : off + w].rearrange(
                "(t p) c -> p t c", p=P
            ),
        )

    # ---- tiles ----
    els = [
        epool.tile([P, NT * w], dt.int16, name=f"el{b}", tag=f"el{b}")
        for b, w in enumerate(W_B)
    ]
    ehs = [
        epool.tile([P, NT * w], dt.int16, name=f"eh{b}", tag=f"eh{b}")
        for b, w in enumerate(W_B)
    ]
    psum = ppool.tile([P, PSCR + 512], dt.float32)
    vis = [
        spool.tile([P, w], dt.int32, name=f"vi{b}", tag=f"vi{b}")
        for b, w in enumerate(W_B)
    ]
    fkw = [
        spool.tile([P, 2 * w], dt.int32, name=f"fkw{b}", tag=f"fkw{b}")
        for b, w in enumerate(W_B)
    ]
    rw = spool.tile([P, 2 * TW], dt.int32, tag="rw")
    acc = spool.tile([P, TW], dt.bfloat16, tag="acc")
    resi = spool.tile([TH, TW], dt.int32, tag="resi")
    st = spool.tile([P, 2 * TW], dt.int32, tag="st")   # smalls scratch t
    sm = spool.tile([P, 2 * TW], dt.int32, tag="sm")   # smalls flag words
    sv = spool.tile([P, TW], dt.int32, tag="sv")       # one-hot V
    svm = spool.tile([P, TW], dt.int32, tag="svm")     # V * M
    ssh = spool.tile([P, TW], dt.int32, tag="ssh")     # VM >> 13

    def encode(plane, b, t0, t1, engine):
        w = W_B[b]
        e = (els if plane == 0 else ehs)[b]
        scale, bias = (-640, 18176) if plane == 0 else (640, 14976)
        xsrc = xs[b][:, t0 * w : t1 * w]
        nc.vector.tensor_scalar(
            e[:, t0 * w : t1 * w],
            xsrc,
            scale,
            bias,
            aop.mult,
            aop.add,
        )

    def mms(b, t0, t1):
        w = W_B[b]
        for t in range(t0, t1):
            for plane, e in ((0, els[b]), (1, ehs[b])):
                base = 64 * plane
                nc.tensor.matmul(
                    psum[base : base + 64, PBANK[b] : PBANK[b] + w],
                    pw[:, 64 * t : 64 * (t + 1)],
                    e[:, t * w : (t + 1) * w].bitcast(dt.bfloat16),
                    start=(t == 0),
                    stop=(t == NT - 1),
                    skip_group_check=True,
                )

    def vi_op(b, engine):
        w = W_B[b]
        nc.scalar.activation(
            vis[b][:],
            psum[:, PBANK[b] : PBANK[b] + w],
            mybir.ActivationFunctionType.Relu,
            bias=0.0,
            scale=1.0,
        )

    def fkws(b):
        w = W_B[b]
        for j, shift in enumerate((5, 10)):
            nc.vector.tensor_scalar(
                fkw[b][:, w * j : w * (j + 1)],
                vis[b][:],
                shift,
                PAIRMASK,
                aop.logical_shift_right,
                aop.bitwise_and,
            )

    def reds(b):
        w = W_B[b]
        nb = NB_B[b]
        for j in range(2):
            with nc.allow_low_precision(reason="small int counts, exact"):
                nc.vector.tensor_reduce(
                    rw[:, TW * j + BOFF[b] : TW * j + BOFF[b] + nb],
                    fkw[b][:, w * j : w * (j + 1)].rearrange(
                        "p (x s) -> p x s", s=SC
                    ),
                    mybir.AxisListType.X,
                    aop.add,
                )

    def smalls(b0, b1):
        # threshold pass over block-columns [b0, b1), fold + output dma
        v = nc.vector

        def w2(t):
            return t[:, :].rearrange("p (j b) -> p j b", j=2)[:, :, b0:b1]

        v.tensor_scalar(w2(st), w2(rw), FLAG_C, None, aop.add)
        v.tensor_scalar(
            w2(sm), w2(st), 9, M_MASK, aop.logical_shift_right, aop.bitwise_and
        )
        # V = m_pair0 + 8 * m_pair1 -> one-hot bits {0,3,10,13}
        v.scalar_tensor_tensor(
            sv[:, b0:b1], sm[:, TW + b0 : TW + b1], 8, sm[:, b0:b1],
            aop.mult, aop.add,
        )
        v.tensor_scalar(svm[:, b0:b1], sv[:, b0:b1], mv[:, 0:1], None, aop.mult)
        v.tensor_scalar(
            ssh[:, b0:b1], svm[:, b0:b1], 13, 7,
            aop.logical_shift_right, aop.bitwise_and,
        )
        v.tensor_copy(acc[:, b0:b1], ssh[:, b0:b1])
        # fold: out[p, b] = acc[p, b] + acc[p+64, b] - 1  (values <= 7, exact)
        nc.tensor.matmul(
            psum[0:64, PFOLD + b0 : PFOLD + b1], pw[:, 512:576], acc[:, b0:b1],
            start=True, stop=True, skip_group_check=True,
        )
        nc.scalar.activation(
            resi[:, b0:b1],
            psum[0:64, PFOLD + b0 : PFOLD + b1],
            mybir.ActivationFunctionType.Copy,
            bias=-1.0,
            scale=1.0,
        )
        nc.sync.dma_start(out_ap[:, b0:b1], resi[:, b0:b1])

    # ---- pipeline emission ----
    warmups()
    for (t0, t1) in CHUNKS[0]:
        encode(0, 0, t0, t1, "dve")
        encode(1, 0, t0, t1, "dve")
        mms(0, t0, t1)
    encode(0, 1, 0, 4, "dve")
    encode(1, 1, 0, 4, "dve")
    mms(1, 0, 4)
    vi_op(0, "act")
    fkws(0)
    reds(0)
    encode(1, 1, 4, 8, "dve")
    encode(0, 1, 4, 8, "dve")
    mms(1, 4, 8)
    encode(1, 2, 0, 8, "dve")
    encode(0, 2, 0, 8, "dve")
    mms(2, 0, 8)
    vi_op(1, "act")
    vi_op(2, "act")
    fkws(1)
    reds(1)
    fkws(2)
    reds(2)
    smalls(0, TW)


def _split_multi_waits(nc):
    """This toolchain's walrus codegen accepts at most ONE semaphore wait per
    engine instruction (two on EventSemaphore).  The Tile scheduler sometimes
    emits more; spill the extras onto same-engine NoOp carriers inserted just
    before the instruction (engines dispatch in order, so the carrier's wait
    is satisfied before the instruction issues -- semantics preserved)."""
    _ensure_path()
    from concourse import mybir

    for func in nc.m.functions:
        for blk in func.blocks:
            insts = blk.instructions
            out = []
            changed = False
            for ins in insts:
                si = ins.sync_info
                cap = 2 if isinstance(ins, mybir.InstEventSemaphore) else 1
                if si and si.on_wait and len(si.on_wait) > cap:
                    waits = list(si.on_wait)
                    for w in waits[:-cap]:
                        out.append(
                            mybir.InstNoOp(
                                name=nc.get_next_instruction_name(),
                                engine=ins.engine,
                                sync_info=mybir.SyncInfo(on_wait=[w], on_update=[]),
                                bass_nofuse=True,
                            )
                        )
                    si.on_wait = waits[-cap:]
                    changed = True
                out.append(ins)
            if changed:
                blk.instructions = out


def _install_ntff_hook():
    """Provide antenv.axon_hooks + the ctypes NTFF profile hook when the
    agent image lacks them (mirrors trn_agent_boot.trn_boot section 6)."""
    import contextlib
    import ctypes
    import types

    try:
        from antenv.axon_hooks import get_axon_ntff_profile_hook  # noqa: F401

        return
    except ImportError:
        pass
    _ensure_path()
    import antenv

    so_path = "/opt/axon/libaxon_pjrt.so"
    try:
        lib = ctypes.CDLL(so_path)
    except OSError:
        return
    if not hasattr(lib, "axon_start_nrt_profile"):
        return
    lib.axon_start_nrt_profile.argtypes = [
        ctypes.POINTER(ctypes.c_int64),
        ctypes.c_size_t,
    ]
    lib.axon_start_nrt_profile.restype = ctypes.c_int64
    lib.axon_stop_nrt_profile.argtypes = [ctypes.c_char_p]
    lib.axon_stop_nrt_profile.restype = ctypes.c_int64

    @contextlib.contextmanager
    def _hook(output_dir, device_ids):
        import jax

        jax.devices()
        if device_ids:
            ids = (ctypes.c_int64 * len(device_ids))(*device_ids)
            rc = lib.axon_start_nrt_profile(ids, len(device_ids))
        else:
            rc = lib.axon_start_nrt_profile(None, 0)
        if rc != 0:
            raise RuntimeError(f"axon_start_nrt_profile rc={rc}")
        try:
            yield
        finally:
            n = lib.axon_stop_nrt_profile(str(output_dir).encode())
            print(f"ntff profile: {n} file(s) written to {output_dir}", file=sys.stderr)

    mod = types.ModuleType("antenv.axon_hooks")
    _h = [_hook]
    mod.set_axon_ntff_profile_hook = lambda h: _h.__setitem__(0, h)
    mod.get_axon_ntff_profile_hook = lambda: _h[0]
    sys.modules["antenv.axon_hooks"] = mod
    antenv.axon_hooks = mod

    # upload_artifacts pushes the NEFF dir to a cloud bucket; keep local.
    from concourse import bass_utils as _bu

    _bu.upload_artifacts = lambda tmpdir: tmpdir


_NC_CACHE = None


def _build_nc(split_waits=True):
    global _NC_CACHE
    if _NC_CACHE is not None:
        return _NC_CACHE
    _ensure_path()
    from contextlib import ExitStack

    import concourse.bass as bass
    import concourse.tile as tile
    from concourse import mybir

    dt = mybir.dt
    nc = bass.Bass("TRN2", target_bir_lowering=False, debug=False)
    label = nc.dram_tensor("label", [H, W], dt.int32, kind="ExternalInput").ap()
    poolw = nc.dram_tensor("poolw", [P, 576], dt.bfloat16, kind="ExternalInput").ap()
    mvt = nc.dram_tensor("mv", [P, 1], dt.float32, kind="ExternalInput").ap()
    out = nc.dram_tensor("out", [TH, TW], dt.int32, kind="ExternalOutput").ap()
    with tile.TileContext(nc) as tc:
        with ExitStack() as ctx:
            emit_downscale(ctx, tc, out, label, poolw, mvt)
    if split_waits:
        _split_multi_waits(nc)
        _NC_CACHE = nc
    return nc


def run_on_hw(label, trace=False):
    """Run on the 8 NeuronCores; returns (out [8,1,64,64] int32, exec_time_ns)."""
    _ensure_path()
    from concourse.bass_utils import run_bass_kernel_spmd

    if trace:
        _install_ntff_hook()
    nc = _build_nc()
    poolw, mv = make_consts()
    label = np.ascontiguousarray(label, dtype=np.int32)
    in_maps = [
        {"label": label[i], "poolw": poolw, "mv": mv} for i in range(N_CORES)
    ]
    r = run_bass_kernel_spmd(nc, in_maps, core_ids=list(range(N_CORES)), trace=trace)
    outs = np.stack([r.results[i]["out"] for i in range(N_CORES)])
    return outs.reshape(8, 1, TH, TW).astype(np.int32), r.exec_time_ns


def kernel(label):
    out, _ = run_on_hw(label, trace=False)
    return out

